# revision 8
# baseline (speedup 1.0000x reference)
"""Trainium2 Bass kernel for the STFT denoiser — fp8 residual formulation.

out = x - iSTFT(ft * c),  c = min(0.1*bias/mag, 1)

iSTFT(STFT(x)) == x exactly (pinv basis + window-sumsquare divide), so the
kernel only computes the small correction delta = iSTFT(ft*c) in fp8
DoubleRow matmuls (2x tensor throughput) and subtracts it from the exact
f32 input.  Per-channel factors 0.1*bias are folded into the host-built
forward/inverse bases, so on-chip pointwise is just:
    msq' = max(reA^2, q2) + imA^2 ;  r = Sr*rsqrt(msq') ;  rec = ft_s*r

Channel packing (1024 = 8 tiles of 128): A-half = re_0..re_511,
B-half = [re_512, im_1..im_511] (zero rows im_0/im_512 dropped; the
partition-0 slot of the first B tile carries re_512, fixed up by tiny
per-partition chains).

8 shards = 4 batches x 2 time-halves, one per NeuronCore.
"""
import sys
for _p in ("/opt/trn_rl_repo", "/root/.axon_site/_ro/trn_rl_repo"):
    if _p not in sys.path:
        sys.path.insert(0, _p)

import numpy as np
import ml_dtypes

import concourse.bass as bass
import concourse.tile as tile
import concourse.mybir as mybir
from concourse import bacc
from concourse.bass_utils import run_bass_kernel_spmd

F32 = mybir.dt.float32
F8 = mybir.dt.float8e4
FP8 = ml_dtypes.float8_e4m3
DR = mybir.MatmulPerfMode.DoubleRow
ALU = mybir.AluOpType

N_FFT = 1024
HOP = 256
CUT = 513
B = 4
T = 1048576
PAD = 512
F_TOTAL = 4097       # global frames

NF = 2052            # frame slots per shard (incl. phantom edges)
NFP = 2064           # REC frame-dim padding (dual-fp8 ldweights needs 16B-aligned ct stride)
W = 342              # frames per forward tile (6 tiles)
NT = 6
XS_LEN = HOP * (NF - 1) + N_FFT          # 526080 input samples per shard
XW = XS_LEN // 128                       # 4110 interleaved words
XWP = 4224                               # padded word count
NGT = 16                                 # inverse g-tiles
DELTA = 3                                # rec column offset

SW = 1024.0          # fwd basis fp8 gain
SX = 32.0            # audio fp8 gain
SR = 128.0           # rec fp8 gain
SI = float(2 ** 21)  # inv basis fp8 gain
RS_SCALE = 1.0 / (SR * SR)               # Rsqrt act input scale
DSC = 1.0 / (SR * SI)                    # psum descale
C_INT = -(8.0 / 3.0) * DSC               # interior -invws/(Sr*Si)

_cache = {}


def _q8(a):
    return np.clip(np.asarray(a, np.float32), -240.0, 240.0).astype(FP8)


def _act(nc, out, in_, func, bias=0.0, scale=1.0):
    """Raw InstActivation emit (Rsqrt is blocked by the bass helper)."""
    eng = nc.scalar
    b = bias if isinstance(bias, bass.AP) else nc.const_aps.scalar_like(bias, in_)
    ins = [eng.lower_ap(in_), eng.lower_ap(b)]
    for arg in (scale, 0.0):
        ins.append(mybir.ImmediateValue(dtype=mybir.dt.float32, value=arg))
    return eng.add_instruction(
        mybir.InstActivation(
            name=nc.get_next_instruction_name(), func=func, ins=ins,
            outs=[eng.lower_ap(out)]))


def _make_host_constants(bias):
    """Bias-dependent fp8 bases + small f32 tables."""
    key = bias.tobytes()
    if _cache.get("bias_key") == key:
        return
    n = np.arange(N_FFT)
    win = 0.5 - 0.5 * np.cos(2.0 * np.pi * n / N_FFT)
    fb = np.fft.fft(np.eye(N_FFT))
    FBm = np.vstack([fb[:CUT].real, fb[:CUT].imag])          # [1026, 1024]
    fwd = FBm * win[None, :]
    inv = np.linalg.pinv(4.0 * FBm).T * win[None, :]         # [1026, 1024]

    b513 = bias.reshape(CUT).astype(np.float64)
    # channel map: slot s in 0..1023 -> (basis row, bias idx)
    # A half (slots 0..511): re_c -> row c ; B half: slot 512 -> re_512
    # (row 512), slots 513.. -> im_c (row 513+c), c = slot-512
    rows = np.empty(1024, np.int64)
    bidx = np.empty(1024, np.int64)
    rows[0:512] = np.arange(512); bidx[0:512] = np.arange(512)
    rows[512] = 512; bidx[512] = 512
    c = np.arange(1, 512)
    rows[512 + c] = 513 + c; bidx[512 + c] = c

    alpha = 0.1 * b513[bidx]                                  # [1024]
    fwd_s = fwd[rows] * (alpha[:, None] * SW)                 # [1024, 1024]
    inv_s = inv[rows] * (alpha[:, None] * SI)                 # [1024, 1024]

    # fwd8[i, kp, e, slot] = fwd_s[slot, 128*(2kp+e)+i]; slots pair-ordered:
    # slot' = 256*pr + (0..127 A: re_{128pr+p} | 128..255 B: b-half tile pr)
    perm = np.empty(1024, np.int64)
    for pr in range(4):
        perm[256 * pr:256 * pr + 128] = np.arange(128 * pr, 128 * pr + 128)
        perm[256 * pr + 128:256 * pr + 256] = 512 + np.arange(
            128 * pr, 128 * pr + 128)
    fwdp = fwd_s[perm]                                        # [1024, 1024]
    fwd8 = np.ascontiguousarray(
        fwdp.T.reshape(8, 128, 1024).transpose(1, 0, 2)
    ).reshape(128, 4, 2, 1024)
    # sanity: fwd8[i, kp, e, s] == fwdp[s, 128*(2kp+e)+i]
    # REC ct order = [A0, A1, A2, A3, B0, B1, B2, B3]
    inve = np.empty((128, 8, 4, 256), np.float64)
    for ct in range(8):
        slot0 = ct * 128 if ct < 4 else 512 + (ct - 4) * 128
        seg = inv_s[slot0:slot0 + 128].reshape(128, 4, 256)
        inve[:, ct] = seg

    # rsqrt bias: r = Rsqrt(msq*RS_SCALE + q2*RS_SCALE) = SR/sqrt(msq+q2)
    q_s = (alpha * SW * SX * 0.1 * b513[bidx]) ** 2           # [1024] q^2
    q2 = (np.maximum(q_s, 1e-30) * RS_SCALE).astype(np.float32)
    q2t = np.zeros((128, 6), np.float32)
    for pr in range(4):
        q2t[:, pr] = q2[128 * pr:128 * pr + 128]              # re-channel q2
    q2t[0, 4] = q2[0]                                         # re_0
    q2t[0, 5] = q2[512]                                       # re_512

    # window sumsquare -> invws/(Sr*Si), edge rows only
    n_len = N_FFT + HOP * (F_TOTAL - 1)
    ws = np.zeros(n_len, np.float64)
    idx = (np.arange(F_TOTAL)[:, None] * HOP + np.arange(N_FFT)[None, :]).ravel()
    np.add.at(ws, idx, np.tile(win ** 2, F_TOTAL))
    tiny = np.finfo(np.float32).tiny
    invws_g = np.where(ws > tiny, 4.0 / ws, 4.0) * DSC

    invws_e = {}
    pmask = {}
    for j in (0, 1):
        Bj = 2048 * j + 2
        arr = np.empty((128, 2, 256), np.float32)
        g = np.arange(128)
        for col, gt in ((0, 0), (1, 15)):
            base = (Bj + 128 * gt + g) * 256
            arr[:, col, :] = invws_g[base[:, None] + np.arange(256)[None, :]]
        invws_e[j] = arr
        pm = np.ones((128, 8, 2), np.float32)
        if j == 0:
            pm[:, :, 0] = 0.0        # zero REC col 0 (phantom f=-1)
        else:
            pm[:, :, 1] = 0.0        # zero REC col 2050 (phantom)
        pmask[j] = pm

    _cache.update(bias_key=key, fwd8=_q8(fwd8), invE8=_q8(inve),
                  q2t=q2t, invws_e=invws_e, pmask=pmask)


def _build_nc():
    if "nc" in _cache:
        return _cache["nc"]
    nc = bacc.Bacc("TRN2", target_bir_lowering=False, debug=False, num_devices=8)

    xs_d = nc.dram_tensor("xs", [128, XWP], F8, kind="ExternalInput")
    fwd8_d = nc.dram_tensor("fwd8", [128, 4, 2, 1024], F8, kind="ExternalInput")
    invE8_d = nc.dram_tensor("invE8", [128, 8, 4, 256], F8, kind="ExternalInput")
    q2t_d = nc.dram_tensor("q2t", [128, 6], F32, kind="ExternalInput")
    invwse_d = nc.dram_tensor("invwse", [128, 2, 256], F32, kind="ExternalInput")
    pmask_d = nc.dram_tensor("pmask", [128, 8, 2], F32, kind="ExternalInput")
    xout_d = nc.dram_tensor("xout", [2048, 256], F32, kind="ExternalInput")
    out_d = nc.dram_tensor("out", [2048, 256], F32, kind="ExternalOutput")

    RSQ = mybir.ActivationFunctionType.Rsqrt

    with tile.TileContext(nc) as tc:
        with (
            tc.tile_pool(name="const", bufs=1) as cpool,
            tc.tile_pool(name="big", bufs=1) as bigp,
            tc.tile_pool(name="tab", bufs=3) as tabp,
            tc.tile_pool(name="sm", bufs=3) as smp,
            tc.tile_pool(name="xo", bufs=4) as xop,
            tc.tile_pool(name="ob", bufs=3) as obp,
            tc.tile_pool(name="psf", bufs=3, space="PSUM") as psf,
            tc.tile_pool(name="psi", bufs=2, space="PSUM") as psi,
        ):
            fwd8 = cpool.tile([128, 4, 2, 1024], F8)
            invE8 = cpool.tile([128, 8, 4, 256], F8)
            q2t = cpool.tile([128, 6], F32)
            invwse = cpool.tile([128, 2, 256], F32)
            pmask = cpool.tile([128, 8, 2], F32)
            # small consts first on SWDGE queue, then fwd8 by pair chunk
            nc.gpsimd.dma_start(q2t[:], q2t_d.ap())
            nc.gpsimd.dma_start(pmask[:], pmask_d.ap())
            nc.gpsimd.dma_start(invwse[:], invwse_d.ap())
            for pr in range(4):
                nc.gpsimd.dma_start(fwd8[:, :, :, 256 * pr:256 * pr + 256],
                                    fwd8_d.ap()[:, :, :, 256 * pr:256 * pr + 256])
            # invE + xout on the Activation HWDGE queue
            for uu in range(2):
                nc.scalar.dma_start(invE8[:, 4 * uu:4 * uu + 4],
                                    invE8_d.ap()[:, 4 * uu:4 * uu + 4])

            X = bigp.tile([128, XWP], F8)
            REC = bigp.tile([128, 8, NFP], F8)

            for c0 in range(0, XWP, 1056):
                nc.sync.dma_start(X[:, c0:c0 + 1056], xs_d.ap()[:, c0:c0 + 1056])

            X4 = X[:].rearrange("p (w e) -> p e w", e=2)      # [128,2,2112]
            RECi = REC[:].rearrange("p (i q) n -> p i q n", i=2)
            out_v = out_d.ap().rearrange("(t g) r -> g t r", g=128)
            xout_v = xout_d.ap().rearrange("(t g) r -> g t r", g=128)

            xo_tiles = {}

            def load_xo(gt):
                xo = xop.tile([128, 256], F32, tag="xo", name="xo")
                nc.scalar.dma_start(xo[:], xout_v[:, gt, :])
                xo_tiles[gt] = xo

            def inverse_gtile(gt):
                q = psi.tile([128, 256], F32, tag="inv", name="q")
                first = True
                for u in range(4):
                    for s in range(4):
                        c0 = 128 * gt + DELTA - s
                        nc.tensor.matmul(
                            q[:, :], REC[:, 2 * u:2 * u + 2, c0:c0 + 128],
                            invE8[:, 2 * u:2 * u + 2, s, :],
                            start=first, stop=(u == 3 and s == 3),
                            perf_mode=DR)
                        first = False
                osb = obp.tile([128, 256], F32, tag="osb", name="osb")
                xo = xo_tiles.pop(gt)
                if gt == 0 or gt == NGT - 1:
                    tmp = obp.tile([128, 256], F32, tag="osb2", name="tmp")
                    wsel = 0 if gt == 0 else 1
                    nc.vector.tensor_tensor(tmp[:], q[:, :],
                                            invwse[:, wsel, :], op=ALU.mult)
                    nc.gpsimd.tensor_tensor(osb[:], xo[:], tmp[:],
                                            op=ALU.subtract)
                else:
                    nc.vector.scalar_tensor_tensor(osb[:], q[:, :], C_INT,
                                                   xo[:], op0=ALU.mult,
                                                   op1=ALU.add)
                nc.sync.dma_start(out_v[:, gt, :], osb[:])

            # prefetch first xout tiles
            for gt in range(3):
                load_xo(gt)

            gt_done = 0
            xo_loaded = 3
            for fti in range(NT):
                m0 = W * fti
                for pr in range(4):
                    pt = psf.tile([128, 2, 512], F32, tag="fwd", name="pt")
                    for half in range(2):
                        lo = 256 * pr + 128 * half
                        for kp in range(4):
                            nc.tensor.matmul(
                                pt[:, half, 0:W],
                                fwd8[:, kp, :, lo:lo + 128],
                                X4[:, :, m0 + kp:m0 + kp + W],
                                start=(kp == 0), stop=(kp == 3),
                                perf_mode=DR)
                    ptv = pt[:, :, 0:W]
                    tAB = tabp.tile([128, 2, W], F32, tag="tab", name="tAB")
                    nc.scalar.square(tAB[:], ptv)
                    msq = smp.tile([128, W], F32, tag="msq", name="msq")
                    nc.gpsimd.tensor_tensor(msq[:], tAB[:, 0, :],
                                            tAB[:, 1, :], op=ALU.add)
                    r = smp.tile([128, W], F32, tag="r", name="r")
                    _act(nc, r[:], msq[:], RSQ, scale=RS_SCALE,
                         bias=q2t[:, pr:pr + 1])
                    rb = r[:].unsqueeze(1).broadcast_to([128, 2, W])
                    nc.vector.tensor_tensor(RECi[:, :, pr, m0:m0 + W], rb,
                                            ptv, op=ALU.mult)
                    if pr == 0:
                        # partition-0 fixups: slots re_0 (A) and re_512 (B)
                        for half, qc in ((0, 4), (1, 5)):
                            r0s = smp.tile([1, W], F32, tag="r0s", name="r0s")
                            _act(nc, r0s[:], tAB[0:1, half, :], RSQ,
                                 scale=RS_SCALE, bias=q2t[0:1, qc:qc + 1])
                            nc.vector.tensor_tensor(
                                REC[0:1, 4 * half, m0:m0 + W], r0s[:],
                                pt[0:1, half, 0:W], op=ALU.mult)
                if fti == 0:
                    nc.vector.tensor_tensor(REC[:, :, 0:1], REC[:, :, 0:1],
                                            pmask[:, :, 0:1], op=ALU.mult)
                elif fti == NT - 1:
                    nc.vector.tensor_tensor(REC[:, :, 2050:2051],
                                            REC[:, :, 2050:2051],
                                            pmask[:, :, 1:2], op=ALU.mult)
                # inverse g-tiles whose rec columns are complete
                avail = m0 + W
                while gt_done < NGT and 128 * gt_done + 131 <= avail:
                    while xo_loaded < min(gt_done + 4, NGT):
                        load_xo(xo_loaded)
                        xo_loaded += 1
                    inverse_gtile(gt_done)
                    gt_done += 1
            while gt_done < NGT:
                while xo_loaded < min(gt_done + 4, NGT):
                    load_xo(xo_loaded)
                    xo_loaded += 1
                inverse_gtile(gt_done)
                gt_done += 1

    nc.compile()
    _cache["nc"] = nc
    return nc


def _prep_inputs(audio, bias_spec):
    bias = np.asarray(bias_spec, dtype=np.float32).reshape(CUT)
    _make_host_constants(bias)

    in_maps = []
    for b in range(B):
        ab = np.asarray(audio[b], dtype=np.float32)
        xp = np.pad(ab, PAD, mode="reflect")
        for j in (0, 1):
            if j == 0:
                xs = np.concatenate([np.zeros(256, np.float32),
                                     xp[0:XS_LEN - 256]])
            else:
                start = HOP * 2047
                xs = np.concatenate([xp[start:], np.zeros(512, np.float32)])
            xsh = np.zeros(XWP * 128, dtype=FP8)
            xsh[:XS_LEN] = _q8(xs * SX)
            xst = np.ascontiguousarray(xsh.reshape(XWP, 128).T)
            xout = np.ascontiguousarray(
                ab[524288 * j:524288 * (j + 1)].reshape(2048, 256))
            in_maps.append({
                "xs": xst,
                "fwd8": _cache["fwd8"],
                "invE8": _cache["invE8"],
                "q2t": _cache["q2t"],
                "invwse": _cache["invws_e"][j],
                "pmask": _cache["pmask"][j],
                "xout": xout,
            })
    return in_maps


def kernel(audio, bias_spec, _trace=False):
    nc = _build_nc()
    in_maps = _prep_inputs(audio, bias_spec)
    res = run_bass_kernel_spmd(nc, in_maps, core_ids=list(range(8)), trace=_trace)
    out = np.empty((B, 1, T), dtype=np.float32)
    for b in range(B):
        for j in (0, 1):
            shard = res.results[2 * b + j]["out"].reshape(-1)
            out[b, 0, 524288 * j: 524288 * (j + 1)] = shard
    if _trace:
        kernel.last_results = res
    return out


# revision 10
# speedup vs baseline: 1.2295x; 1.2295x over previous
"""Trainium2 Bass kernel for the STFT denoiser — fp8 residual formulation.

out = x - iSTFT(ft * c),  c = min(0.1*bias/mag, 1)

iSTFT(STFT(x)) == x exactly (pinv basis + window-sumsquare divide), so the
kernel only computes the small correction delta = iSTFT(ft*c) in fp8
DoubleRow matmuls (2x tensor throughput) and subtracts it from the exact
f32 input.  Per-channel factors 0.1*bias are folded into the host-built
forward/inverse bases, so on-chip pointwise is just:
    msq' = max(reA^2, q2) + imA^2 ;  r = Sr*rsqrt(msq') ;  rec = ft_s*r

Channel packing (1024 = 8 tiles of 128): A-half = re_0..re_511,
B-half = [re_512, im_1..im_511] (zero rows im_0/im_512 dropped; the
partition-0 slot of the first B tile carries re_512, fixed up by tiny
per-partition chains).

8 shards = 4 batches x 2 time-halves, one per NeuronCore.
"""
import sys
for _p in ("/opt/trn_rl_repo", "/root/.axon_site/_ro/trn_rl_repo"):
    if _p not in sys.path:
        sys.path.insert(0, _p)

import numpy as np
import ml_dtypes

import concourse.bass as bass
import concourse.tile as tile
import concourse.mybir as mybir
from concourse import bacc
from concourse.bass_utils import run_bass_kernel_spmd

F32 = mybir.dt.float32
F8 = mybir.dt.float8e4
FP8 = ml_dtypes.float8_e4m3
DR = mybir.MatmulPerfMode.DoubleRow
ALU = mybir.AluOpType

N_FFT = 1024
HOP = 256
CUT = 513
B = 4
T = 1048576
PAD = 512
F_TOTAL = 4097       # global frames

NF = 2052            # frame slots per shard (incl. phantom edges)
NFP = 2064           # REC frame-dim padding (dual-fp8 ldweights needs 16B-aligned ct stride)
W = 342              # frames per forward tile (6 tiles)
NT = 6
XS_LEN = HOP * (NF - 1) + N_FFT          # 526080 input samples per shard
XW = XS_LEN // 128                       # 4110 interleaved words
XWP = 4224                               # padded word count
NGT = 16                                 # inverse g-tiles
DELTA = 3                                # rec column offset

SW = 1024.0          # fwd basis fp8 gain
SX = 32.0            # audio fp8 gain
SR = 128.0           # rec fp8 gain
SI = float(2 ** 21)  # inv basis fp8 gain
RS_SCALE = 1.0 / (SR * SR)               # Rsqrt act input scale
DSC = 1.0 / (SR * SI)                    # psum descale
C_INT = -(8.0 / 3.0) * DSC               # interior -invws/(Sr*Si)

_cache = {}


def _q8(a):
    return np.clip(np.asarray(a, np.float32), -240.0, 240.0).astype(FP8)


def _act(nc, out, in_, func, bias=0.0, scale=1.0):
    """Raw InstActivation emit (Rsqrt is blocked by the bass helper)."""
    eng = nc.scalar
    b = bias if isinstance(bias, bass.AP) else nc.const_aps.scalar_like(bias, in_)
    ins = [eng.lower_ap(in_), eng.lower_ap(b)]
    for arg in (scale, 0.0):
        ins.append(mybir.ImmediateValue(dtype=mybir.dt.float32, value=arg))
    return eng.add_instruction(
        mybir.InstActivation(
            name=nc.get_next_instruction_name(), func=func, ins=ins,
            outs=[eng.lower_ap(out)]))


def _make_host_constants(bias):
    """Bias-dependent fp8 bases + small f32 tables."""
    key = bias.tobytes()
    if _cache.get("bias_key") == key:
        return
    n = np.arange(N_FFT)
    win = 0.5 - 0.5 * np.cos(2.0 * np.pi * n / N_FFT)
    fb = np.fft.fft(np.eye(N_FFT))
    FBm = np.vstack([fb[:CUT].real, fb[:CUT].imag])          # [1026, 1024]
    fwd = FBm * win[None, :]
    inv = np.linalg.pinv(4.0 * FBm).T * win[None, :]         # [1026, 1024]

    b513 = bias.reshape(CUT).astype(np.float64)
    # channel map: slot s in 0..1023 -> (basis row, bias idx)
    # A half (slots 0..511): re_c -> row c ; B half: slot 512 -> re_512
    # (row 512), slots 513.. -> im_c (row 513+c), c = slot-512
    rows = np.empty(1024, np.int64)
    bidx = np.empty(1024, np.int64)
    rows[0:512] = np.arange(512); bidx[0:512] = np.arange(512)
    rows[512] = 512; bidx[512] = 512
    c = np.arange(1, 512)
    rows[512 + c] = 513 + c; bidx[512 + c] = c

    alpha = 0.1 * b513[bidx]                                  # [1024]
    fwd_s = fwd[rows] * (alpha[:, None] * SW)                 # [1024, 1024]
    inv_s = inv[rows] * (alpha[:, None] * SI)                 # [1024, 1024]

    # fwd8[i, kp, e, slot] = fwd_s[slot, 128*(2kp+e)+i]; slots pair-ordered:
    # slot' = 256*pr + (0..127 A: re_{128pr+p} | 128..255 B: b-half tile pr)
    perm = np.empty(1024, np.int64)
    for pr in range(4):
        perm[256 * pr:256 * pr + 128] = np.arange(128 * pr, 128 * pr + 128)
        perm[256 * pr + 128:256 * pr + 256] = 512 + np.arange(
            128 * pr, 128 * pr + 128)
    fwdp = fwd_s[perm]                                        # [1024, 1024]
    fwd8 = np.ascontiguousarray(
        fwdp.T.reshape(8, 128, 1024).transpose(1, 0, 2)
    ).reshape(128, 4, 2, 1024)
    # sanity: fwd8[i, kp, e, s] == fwdp[s, 128*(2kp+e)+i]
    # REC ct order = [A0, A1, A2, A3, B0, B1, B2, B3]
    inve = np.empty((128, 8, 4, 256), np.float64)
    for ct in range(8):
        slot0 = ct * 128 if ct < 4 else 512 + (ct - 4) * 128
        seg = inv_s[slot0:slot0 + 128].reshape(128, 4, 256)
        inve[:, ct] = seg

    # rsqrt bias: r = Rsqrt(msq*RS_SCALE + q2*RS_SCALE) = SR/sqrt(msq+q2)
    q_s = (alpha * SW * SX * 0.1 * b513[bidx]) ** 2           # [1024] q^2
    q2 = (np.maximum(q_s, 1e-30) * RS_SCALE).astype(np.float32)
    q2t = np.zeros((128, 6), np.float32)
    for pr in range(4):
        q2t[:, pr] = q2[128 * pr:128 * pr + 128]              # re-channel q2
    q2t[0, 4] = q2[0]                                         # re_0
    q2t[0, 5] = q2[512]                                       # re_512

    # window sumsquare -> invws/(Sr*Si), edge rows only
    n_len = N_FFT + HOP * (F_TOTAL - 1)
    ws = np.zeros(n_len, np.float64)
    idx = (np.arange(F_TOTAL)[:, None] * HOP + np.arange(N_FFT)[None, :]).ravel()
    np.add.at(ws, idx, np.tile(win ** 2, F_TOTAL))
    tiny = np.finfo(np.float32).tiny
    invws_g = np.where(ws > tiny, 4.0 / ws, 4.0) * DSC

    invws_e = {}
    pmask = {}
    for j in (0, 1):
        Bj = 2048 * j + 2
        arr = np.empty((128, 2, 256), np.float32)
        g = np.arange(128)
        for col, gt in ((0, 0), (1, 15)):
            base = (Bj + 128 * gt + g) * 256
            arr[:, col, :] = invws_g[base[:, None] + np.arange(256)[None, :]]
        invws_e[j] = arr
        pm = np.ones((128, 8, 2), np.float32)
        if j == 0:
            pm[:, :, 0] = 0.0        # zero REC col 0 (phantom f=-1)
        else:
            pm[:, :, 1] = 0.0        # zero REC col 2050 (phantom)
        pmask[j] = pm

    _cache.update(bias_key=key, fwd8=_q8(fwd8), invE8=_q8(inve),
                  q2t=q2t, invws_e=invws_e, pmask=pmask)


def _build_nc():
    if "nc" in _cache:
        return _cache["nc"]
    nc = bacc.Bacc("TRN2", target_bir_lowering=False, debug=False, num_devices=8)

    xs_d = nc.dram_tensor("xs", [128, XWP], F8, kind="ExternalInput")
    fwd8_d = nc.dram_tensor("fwd8", [128, 4, 2, 1024], F8, kind="ExternalInput")
    invE8_d = nc.dram_tensor("invE8", [128, 8, 4, 256], F8, kind="ExternalInput")
    q2t_d = nc.dram_tensor("q2t", [128, 6], F32, kind="ExternalInput")
    invwse_d = nc.dram_tensor("invwse", [128, 2, 256], F32, kind="ExternalInput")
    pmask_d = nc.dram_tensor("pmask", [128, 8, 2], F32, kind="ExternalInput")
    xout_d = nc.dram_tensor("xout", [2048, 256], F32, kind="ExternalInput")
    out_d = nc.dram_tensor("out", [2048, 256], F32, kind="ExternalOutput")

    RSQ = mybir.ActivationFunctionType.Rsqrt

    with tile.TileContext(nc) as tc:
        with (
            tc.tile_pool(name="const", bufs=1) as cpool,
            tc.tile_pool(name="big", bufs=1) as bigp,
            tc.tile_pool(name="tab", bufs=3) as tabp,
            tc.tile_pool(name="sm", bufs=3) as smp,
            tc.tile_pool(name="xo", bufs=4) as xop,
            tc.tile_pool(name="ob", bufs=3) as obp,
            tc.tile_pool(name="psf", bufs=3, space="PSUM") as psf,
            tc.tile_pool(name="psi", bufs=2, space="PSUM") as psi,
        ):
            fwd8 = cpool.tile([128, 4, 2, 1024], F8)
            invE8 = cpool.tile([128, 8, 4, 256], F8)
            q2t = cpool.tile([128, 6], F32)
            invwse = cpool.tile([128, 2, 256], F32)
            pmask = cpool.tile([128, 8, 2], F32)
            # fwd8 first on the fast Activation HWDGE queue (gates first
            # matmul), then invE; small consts go down SWDGE
            for pr in range(4):
                nc.scalar.dma_start(fwd8[:, :, :, 256 * pr:256 * pr + 256],
                                    fwd8_d.ap()[:, :, :, 256 * pr:256 * pr + 256])
            for uu in range(2):
                nc.scalar.dma_start(invE8[:, 4 * uu:4 * uu + 4],
                                    invE8_d.ap()[:, 4 * uu:4 * uu + 4])
            nc.gpsimd.dma_start(q2t[:], q2t_d.ap())
            nc.gpsimd.dma_start(pmask[:], pmask_d.ap())
            nc.gpsimd.dma_start(invwse[:], invwse_d.ap())

            X = bigp.tile([128, XWP], F8)
            REC = bigp.tile([128, 8, NFP], F8)

            for c0 in range(0, XWP, 1056):
                nc.sync.dma_start(X[:, c0:c0 + 1056], xs_d.ap()[:, c0:c0 + 1056])

            X4 = X[:].rearrange("p (w e) -> p e w", e=2)      # [128,2,2112]
            RECi = REC[:].rearrange("p (i q) n -> p i q n", i=2)
            out_v = out_d.ap().rearrange("(t g) r -> g t r", g=128)
            xout_v = xout_d.ap().rearrange("(t g) r -> g t r", g=128)

            xo_tiles = {}

            def load_xo(gt):
                xo = xop.tile([128, 256], F32, tag="xo", name="xo")
                nc.sync.dma_start(xo[:], xout_v[:, gt, :])
                xo_tiles[gt] = xo

            def inverse_gtile(gt):
                q = psi.tile([128, 256], F32, tag="inv", name="q")
                first = True
                for u in range(4):
                    for s in range(4):
                        c0 = 128 * gt + DELTA - s
                        nc.tensor.matmul(
                            q[:, :], REC[:, 2 * u:2 * u + 2, c0:c0 + 128],
                            invE8[:, 2 * u:2 * u + 2, s, :],
                            start=first, stop=(u == 3 and s == 3),
                            perf_mode=DR)
                        first = False
                osb = obp.tile([128, 256], F32, tag="osb", name="osb")
                xo = xo_tiles.pop(gt)
                if gt == 0 or gt == NGT - 1:
                    tmp = obp.tile([128, 256], F32, tag="osb2", name="tmp")
                    wsel = 0 if gt == 0 else 1
                    nc.vector.tensor_tensor(tmp[:], q[:, :],
                                            invwse[:, wsel, :], op=ALU.mult)
                    nc.gpsimd.tensor_tensor(osb[:], xo[:], tmp[:],
                                            op=ALU.subtract)
                else:
                    nc.vector.scalar_tensor_tensor(osb[:], q[:, :], C_INT,
                                                   xo[:], op0=ALU.mult,
                                                   op1=ALU.add)
                nc.sync.dma_start(out_v[:, gt, :], osb[:])

            # prefetch first xout tiles
            for gt in range(3):
                load_xo(gt)

            gt_done = 0
            xo_loaded = 3
            for fti in range(NT):
                m0 = W * fti
                for pr in range(4):
                    pt = psf.tile([128, 2, 512], F32, tag="fwd", name="pt")
                    for half in range(2):
                        lo = 256 * pr + 128 * half
                        for kp in range(4):
                            nc.tensor.matmul(
                                pt[:, half, 0:W],
                                fwd8[:, kp, :, lo:lo + 128],
                                X4[:, :, m0 + kp:m0 + kp + W],
                                start=(kp == 0), stop=(kp == 3),
                                perf_mode=DR)
                    ptv = pt[:, :, 0:W]
                    tAB = tabp.tile([128, 2, W], F32, tag="tab", name="tAB")
                    nc.scalar.square(tAB[:], ptv)
                    msq = smp.tile([128, W], F32, tag="msq", name="msq")
                    nc.gpsimd.tensor_tensor(msq[:], tAB[:, 0, :],
                                            tAB[:, 1, :], op=ALU.add)
                    r = smp.tile([128, W], F32, tag="r", name="r")
                    _act(nc, r[:], msq[:], RSQ, scale=RS_SCALE,
                         bias=q2t[:, pr:pr + 1])
                    rb = r[:].unsqueeze(1).broadcast_to([128, 2, W])
                    nc.vector.tensor_tensor(RECi[:, :, pr, m0:m0 + W], rb,
                                            ptv, op=ALU.mult)
                    if pr == 0:
                        # partition-0 fixups: slots re_0 (A) and re_512 (B)
                        for half, qc in ((0, 4), (1, 5)):
                            r0s = smp.tile([1, W], F32, tag="r0s", name="r0s")
                            _act(nc, r0s[:], tAB[0:1, half, :], RSQ,
                                 scale=RS_SCALE, bias=q2t[0:1, qc:qc + 1])
                            nc.vector.tensor_tensor(
                                REC[0:1, 4 * half, m0:m0 + W], r0s[:],
                                pt[0:1, half, 0:W], op=ALU.mult)
                if fti == 0:
                    nc.vector.tensor_tensor(REC[:, :, 0:1], REC[:, :, 0:1],
                                            pmask[:, :, 0:1], op=ALU.mult)
                elif fti == NT - 1:
                    nc.vector.tensor_tensor(REC[:, :, 2050:2051],
                                            REC[:, :, 2050:2051],
                                            pmask[:, :, 1:2], op=ALU.mult)
                # inverse g-tiles whose rec columns are complete
                avail = m0 + W
                while gt_done < NGT and 128 * gt_done + 131 <= avail:
                    while xo_loaded < min(gt_done + 4, NGT):
                        load_xo(xo_loaded)
                        xo_loaded += 1
                    inverse_gtile(gt_done)
                    gt_done += 1
            while gt_done < NGT:
                while xo_loaded < min(gt_done + 4, NGT):
                    load_xo(xo_loaded)
                    xo_loaded += 1
                inverse_gtile(gt_done)
                gt_done += 1

    nc.compile()
    _cache["nc"] = nc
    return nc


def _prep_inputs(audio, bias_spec):
    bias = np.asarray(bias_spec, dtype=np.float32).reshape(CUT)
    _make_host_constants(bias)

    in_maps = []
    for b in range(B):
        ab = np.asarray(audio[b], dtype=np.float32)
        xp = np.pad(ab, PAD, mode="reflect")
        for j in (0, 1):
            if j == 0:
                xs = np.concatenate([np.zeros(256, np.float32),
                                     xp[0:XS_LEN - 256]])
            else:
                start = HOP * 2047
                xs = np.concatenate([xp[start:], np.zeros(512, np.float32)])
            xsh = np.zeros(XWP * 128, dtype=FP8)
            xsh[:XS_LEN] = _q8(xs * SX)
            xst = np.ascontiguousarray(xsh.reshape(XWP, 128).T)
            xout = np.ascontiguousarray(
                ab[524288 * j:524288 * (j + 1)].reshape(2048, 256))
            in_maps.append({
                "xs": xst,
                "fwd8": _cache["fwd8"],
                "invE8": _cache["invE8"],
                "q2t": _cache["q2t"],
                "invwse": _cache["invws_e"][j],
                "pmask": _cache["pmask"][j],
                "xout": xout,
            })
    return in_maps


def kernel(audio, bias_spec, _trace=False):
    nc = _build_nc()
    in_maps = _prep_inputs(audio, bias_spec)
    res = run_bass_kernel_spmd(nc, in_maps, core_ids=list(range(8)), trace=_trace)
    out = np.empty((B, 1, T), dtype=np.float32)
    for b in range(B):
        for j in (0, 1):
            shard = res.results[2 * b + j]["out"].reshape(-1)
            out[b, 0, 524288 * j: 524288 * (j + 1)] = shard
    if _trace:
        kernel.last_results = res
    return out
